# revision 17
# baseline (speedup 1.0000x reference)
"""Trainium2 Bass kernel for nn_C_Cross_Attention3D (B=16, C=768, H=W=64, HEADS=12).

Math (per batch b):
  q   = l2norm_per_head(Wq @ y_b + bq)                      # [12, 64]
  k   = Wk @ x_b + bk                                       # [768, N], N = 4096
  s   = (Qbd^T k) / max(||k||_head, eps)                    # [12, N] cosine scores
  a   = softmax_N(s)                                        # [12, N]
  out = Wp @ (Wv @ (x_b @ a^T |head-diag) + bv) + bp        # [768]

Key restructuring vs. the reference: the V projection commutes with the
attention pooling (one query token per head), so instead of projecting all
N tokens through Wv we pool x with the attention weights first:
  out_attn[head h] = Wv[h_rows, :] @ (x @ a_h^T)  + bv
This halves the dominant GEMM (only K projection runs over all tokens).

Transposes (x^T for the pooling contraction, a^T) are done by DMA-transpose
through a DRAM bounce buffer in bf16, keeping the PE free for matmuls.

Distribution: pure data-parallel over batch, 2 batches per core, 8 cores.
No collectives; host scatters inputs / gathers outputs.

Self-contained: hardcodes all shapes; no sibling imports.
"""

import numpy as np
import ml_dtypes

import concourse.bass as bass
import concourse.mybir as mybir
import concourse.tile as tile
from concourse import bacc
from concourse.bass import ts
from concourse.bass_utils import run_bass_kernel_spmd
from concourse.masks import make_identity

F32 = mybir.dt.float32
BF16 = mybir.dt.bfloat16
AF = mybir.ActivationFunctionType
OP = mybir.AluOpType
AX = mybir.AxisListType

B, C, HEADS, HD = 16, 768, 12, 64
N = 64 * 64                 # tokens per batch
NCORES = 8
BPC = B // NCORES           # batches per core = 2
CT = C // 128               # 6 c-tiles (contraction / channel tiles)
FT = 512                    # token f-tile size
NFT = N // FT               # 8 f-tiles
NNT = N // 128              # 32 n-tiles of 128 tokens
NCH = 4                     # x^T DMA-transpose chunks per batch
CHW = N // NCH              # chunk width in tokens (1024)
EPS = 1e-12


def _act_table_filter():
    """Restrict activation-table choice to the single set that covers all
    funcs this kernel uses (Copy/Exp/Ln/Square), so no mid-kernel
    ACT_TABLE_LOAD swaps are emitted. Index positions are preserved."""
    import functools
    import concourse.bacc as _bacc

    orig = _bacc.get_activation_tables

    @functools.cache
    def filtered(arch):
        t = orig(arch)
        return {
            name: (s if name == "natural_log_exp_and_others" else set())
            for name, s in t.items()
        }

    return orig, filtered


def _build_nc():
    nc = bacc.Bacc(
        "TRN2",
        target_bir_lowering=False,
        debug=False,
        enable_asserts=False,
        num_devices=NCORES,
    )

    x_d = nc.dram_tensor("x", [BPC, C, N], F32, kind="ExternalInput").ap()
    y_d = nc.dram_tensor("y", [C, BPC], F32, kind="ExternalInput").ap()
    wk_d = nc.dram_tensor("wkT", [C, C], BF16, kind="ExternalInput").ap()
    wk2_d = nc.dram_tensor("wk2", [C, C], BF16, kind="ExternalInput").ap()
    wq_d = nc.dram_tensor("wqT", [C, C], BF16, kind="ExternalInput").ap()
    wv_d = nc.dram_tensor("wvT", [C, C], BF16, kind="ExternalInput").ap()
    wp_d = nc.dram_tensor("wpT", [C, C], BF16, kind="ExternalInput").ap()
    bq_d = nc.dram_tensor("bq", [C], F32, kind="ExternalInput").ap()
    bk_d = nc.dram_tensor("bk", [C], F32, kind="ExternalInput").ap()
    bpz_d = nc.dram_tensor("bpz", [C], F32, kind="ExternalInput").ap()
    z_d = nc.dram_tensor("z", [C, BPC], F32, kind="ExternalOutput").ap()

    with tile.TileContext(nc) as tc:
        _emit(nc, tc, x_d, y_d, wk_d, wk2_d, wq_d, wv_d, wp_d, bq_d, bk_d, bpz_d,
              z_d)
    import concourse.bacc as _bacc
    orig, filtered = _act_table_filter()
    _bacc.get_activation_tables = filtered
    try:
        nc.compile()
    finally:
        _bacc.get_activation_tables = orig
    return nc


def _emit(nc, tc, x_d, y_d, wk_d, wk2_d, wq_d, wv_d, wp_d, bq_d, bk_d, bpz_d,
          z_d):
    from contextlib import ExitStack

    ctx = ExitStack()
    with ctx:
        const = ctx.enter_context(tc.tile_pool(name="const", bufs=1))
        statics = ctx.enter_context(tc.tile_pool(name="statics", bufs=1))
        xf_pool = ctx.enter_context(tc.tile_pool(name="xf", bufs=2))
        xb_pool = ctx.enter_context(tc.tile_pool(name="xb", bufs=2))
        k2_pool = ctx.enter_context(tc.tile_pool(name="k2", bufs=2))
        small = ctx.enter_context(tc.tile_pool(name="small", bufs=4))
        at_pool = ctx.enter_context(tc.tile_pool(name="at", bufs=2))
        dram = ctx.enter_context(tc.tile_pool(name="dram", bufs=6, space="DRAM"))
        kp_pool = ctx.enter_context(tc.tile_pool(name="kp", bufs=6, space="PSUM"))
        pp_pool = ctx.enter_context(tc.tile_pool(name="pp", bufs=2, space="PSUM"))

        # ---- constants / weights -------------------------------------------
        id128_f = const.tile([128, 128], F32)
        make_identity(nc, id128_f)
        id64_f = const.tile([64, 64], F32)
        make_identity(nc, id64_f)

        wk_sb = const.tile([128, CT, C], BF16)
        nc.sync.dma_start(wk_sb, wk_d.rearrange("(c p) o -> p c o", p=128))
        wk2_sb = const.tile([128, CT, C], BF16)
        nc.sync.dma_start(wk2_sb, wk2_d.rearrange("(o p) c -> p o c", p=128))
        wq_sb = const.tile([128, CT, C], BF16)
        nc.sync.dma_start(wq_sb, wq_d.rearrange("(c p) o -> p c o", p=128))
        bq_sb = const.tile([128, CT], F32)
        nc.sync.dma_start(bq_sb, bq_d.rearrange("(c p) -> p c", p=128))
        bk_sb = const.tile([128, CT], F32)
        nc.sync.dma_start(bk_sb, bk_d.rearrange("(c p) -> p c", p=128))
        bpz_sb = const.tile([128, CT], F32)
        nc.sync.dma_start(bpz_sb, bpz_d.rearrange("(c p) -> p c", p=128))

        # ones_bd[c, h] = 1 if c // 64 == h  (block-diagonal head indicator)
        ones_bf = const.tile([128, CT, HEADS], BF16)
        ones_f = const.tile([128, CT, HEADS], F32)
        onesT_f = const.tile([HEADS, C], F32)
        nc.vector.memset(ones_bf, 0.0)
        nc.vector.memset(ones_f, 0.0)
        for c in range(CT):
            for half in range(2):
                h = 2 * c + half
                rows = slice(64 * half, 64 * (half + 1))
                nc.vector.memset(ones_bf[rows, c, h : h + 1], 1.0)
                nc.vector.memset(ones_f[rows, c, h : h + 1], 1.0)
        for c in range(CT):
            otp = kp_pool.tile([HEADS, 128], F32, tag="kp")
            nc.tensor.transpose(otp, ones_f[:, c, :], id128_f)
            nc.scalar.copy(out=onesT_f[:, ts(c, 128)], in_=otp)

        # ---- statics --------------------------------------------------------
        scores_all = statics.tile([64, N], F32)   # rows: 32*b + h (12 per batch)
        nc.vector.memset(scores_all, 0.0)
        xT_all = statics.tile([128, NNT, C], BF16)  # transposed x, current batch
        pooledT_all = statics.tile([64, C], F32)
        nc.vector.memset(pooledT_all, 0.0)

        # ---- q path (both batches at once) ---------------------------------
        y_sb = const.tile([128, CT, BPC], F32)
        nc.sync.dma_start(y_sb, y_d.rearrange("(c p) b -> p c b", p=128))
        y_bf = const.tile([128, CT, BPC], BF16)
        nc.vector.tensor_copy(out=y_bf, in_=y_sb)

        q_sb = const.tile([128, CT, BPC], F32)
        for o in range(CT):
            qp = kp_pool.tile([128, BPC], F32, tag="kp")
            for c in range(CT):
                nc.tensor.matmul(
                    qp, wq_sb[:, c, ts(o, 128)], y_bf[:, c, :],
                    start=(c == 0), stop=(c == CT - 1),
                )
            nc.vector.tensor_tensor(
                out=q_sb[:, o, :], in0=qp,
                in1=bq_sb[:, o, None].to_broadcast((128, BPC)), op=OP.add,
            )
        q2_sb = const.tile([128, CT, BPC], F32)
        nc.scalar.activation(out=q2_sb, in_=q_sb, func=AF.Square)
        ssqq = kp_pool.tile([HEADS, BPC], F32, tag="kp")
        for c in range(CT):
            nc.tensor.matmul(
                ssqq, ones_f[:, c, :], q2_sb[:, c, :],
                start=(c == 0), stop=(c == CT - 1),
            )
        rq = const.tile([HEADS, BPC], F32)
        nc.scalar.activation(out=rq, in_=ssqq, func=AF.Ln)
        nc.scalar.activation(out=rq, in_=rq, func=AF.Exp, scale=-0.5)
        nc.vector.tensor_scalar_min(rq, rq, 1.0 / EPS)
        # broadcast rq back to channel layout via block-diag ones matmul
        rqbc = kp_pool.tile([128, CT, BPC], F32, tag="kp")
        for c in range(CT):
            nc.tensor.matmul(
                rqbc[:, c, :], onesT_f[:, ts(c, 128)], rq,
                start=(c == 0), stop=(c == CT - 1), skip_group_check=True,
            )
        qn_sb = const.tile([128, CT, BPC], F32)
        nc.vector.tensor_tensor(out=qn_sb, in0=q_sb, in1=rqbc, op=OP.mult)
        # scatter into block-diagonal Qbd [c, 32*b + h]
        qbd_f = const.tile([128, CT, 32 * BPC], F32)
        nc.vector.memset(qbd_f, 0.0)
        for c in range(CT):
            for half in range(2):
                h = 2 * c + half
                rows = slice(64 * half, 64 * (half + 1))
                for b in range(BPC):
                    col = 32 * b + h
                    nc.vector.tensor_copy(
                        out=qbd_f[rows, c, col : col + 1],
                        in_=qn_sb[rows, c, b : b + 1],
                    )
        qbd_bf = const.tile([128, CT, 32 * BPC], BF16)
        nc.vector.tensor_copy(out=qbd_bf, in_=qbd_f)
        # fold q into the K projection: raw scores = (Wk^T Qbd)^T x + Qbd^T bk
        wtld_bf = const.tile([128, CT, 32 * BPC], BF16)
        for m in range(CT):
            wtp = kp_pool.tile([128, 32 * BPC], F32, tag="kp")
            for ot in range(CT):
                nc.tensor.matmul(
                    wtp, wk2_sb[:, ot, ts(m, 128)], qbd_bf[:, ot, :],
                    start=(ot == 0), stop=(ot == CT - 1),
                )
            nc.vector.tensor_copy(out=wtld_bf[:, m, :], in_=wtp)
        qbk_sb = const.tile([32 * BPC, 1], F32)
        qbkp = kp_pool.tile([32 * BPC, 1], F32, tag="kp")
        for ot in range(CT):
            nc.tensor.matmul(
                qbkp, qbd_f[:, ot, :], bk_sb[:, ot, None],
                start=(ot == 0), stop=(ot == CT - 1),
            )
        nc.vector.tensor_copy(out=qbk_sb, in_=qbkp)

        # ---- main per-batch ------------------------------------------------
        for b in range(BPC):
            R = slice(32 * b, 32 * b + HEADS)
            x_b = x_d[b].rearrange("(c p) n -> p c n", p=128)
            nmx8 = small.tile([64, NFT], F32, tag="nmx8")

            for i in range(NFT):
                xf = xf_pool.tile([128, CT, FT], F32)
                nc.sync.dma_start(xf, x_b[:, :, ts(i, FT)])
                xb = xb_pool.tile([128, CT, FT], BF16)
                nc.vector.tensor_copy(out=xb, in_=xf)
                # SBUF->SBUF XBAR transpose straight into xT_all
                for c in range(CT):
                    nc.scalar.dma_start_transpose(
                        xT_all[:, 4 * i : 4 * i + 4, ts(c, 128)],
                        xb[:, c, :],
                    )

                k2sb = k2_pool.tile([128, CT, FT], BF16)
                for o in range(CT):
                    kp = kp_pool.tile([128, FT], F32, tag="kp")
                    for c in range(CT):
                        nc.tensor.matmul(
                            kp, wk_sb[:, c, ts(o, 128)], xb[:, c, :],
                            start=(c == 0), stop=(c == CT - 1),
                        )
                    nc.scalar.activation(
                        out=k2sb[:, o, :], in_=kp, func=AF.Square,
                        bias=bk_sb[:, o : o + 1], scale=1.0,
                    )

                # sp on col groups 0-1, sq on col group 2 (partition base 64)
                # of a separate bank: interleaved matmuls run concurrently on
                # disjoint column groups of the PE array.
                spt = kp_pool.tile([32 * BPC, FT], F32, tag="kp")
                sqt = kp_pool.tile([64 + HEADS, FT], F32, tag="kp")
                sp = spt[:]
                sq = sqt[64 : 64 + HEADS]
                for c in range(CT):
                    nc.tensor.matmul(
                        sp, wtld_bf[:, c, :], xb[:, c, :],
                        start=(c == 0), stop=(c == CT - 1), skip_group_check=True,
                    )
                    nc.tensor.matmul(
                        sq, ones_bf[:, c, :], k2sb[:, c, :],
                        start=(c == 0), stop=(c == CT - 1), skip_group_check=True,
                    )
                # r = ssq^-1/2 (clamped to 1/eps); scores = (raw + qbk) * r
                rt = small.tile([HEADS, FT], F32, tag="rt")
                nc.scalar.activation(out=rt, in_=sq, func=AF.Ln)
                nc.scalar.activation(out=rt, in_=rt, func=AF.Exp, scale=-0.5)
                nc.vector.tensor_scalar_min(rt, rt, 1.0 / EPS)
                nc.vector.tensor_scalar(
                    out=sp[R, :], in0=sp[R, :],
                    scalar1=qbk_sb[R], scalar2=None, op0=OP.add,
                )
                nc.vector.tensor_tensor(
                    out=scores_all[R, ts(i, FT)], in0=sp[R, :], in1=rt, op=OP.mult,
                )
                nc.vector.tensor_reduce(
                    nmx8[R, i : i + 1], scores_all[R, ts(i, FT)],
                    axis=AX.X, op=OP.max)

            # softmax over N for this batch's 12 head-rows; exp goes straight
            # to the bf16 bounce tile, normalization is folded into the
            # pooled eviction (pooled /= sum(exp)).
            nmx = small.tile([64, 1], F32, tag="st")
            nc.vector.tensor_reduce(
                nmx[R], nmx8[R, :], axis=AX.X, op=OP.max, negate=True)
            se = small.tile([64, 1], F32, tag="se")
            nc.vector.memset(se[R], 0.0)
            ab = at_pool.tile([64, N], BF16, tag="ab")
            nc.scalar.activation(
                out=ab[R, :], in_=scores_all[R, :], func=AF.Exp,
                bias=nmx[R], scale=1.0, accum_out=se[R],
            )
            rse = small.tile([64, 1], F32, tag="st")
            nc.vector.reciprocal(rse[R], se[R])

            # attn SBUF->SBUF transposed  [128, NNT, 32]
            attnT = at_pool.tile([128, NNT, 32], BF16, tag="attnT")
            nc.scalar.dma_start_transpose(attnT, ab[32 * b : 32 * b + 32, :])

            # attn-weighted pooling of x: pooledT = a @ x^T   [12, 768]
            ppt0 = pp_pool.tile([HEADS, 384], F32, tag="pp")
            ppt1 = pp_pool.tile([32 + HEADS, 384], F32, tag="pp")
            pp0 = ppt0[:]
            pp1 = ppt1[32 : 32 + HEADS]
            for nt in range(NNT):
                nc.tensor.matmul(
                    pp0, attnT[:, nt, 0:HEADS], xT_all[:, nt, 0:384],
                    start=(nt == 0), stop=(nt == NNT - 1), skip_group_check=True,
                )
                nc.tensor.matmul(
                    pp1, attnT[:, nt, 0:HEADS], xT_all[:, nt, 384:768],
                    start=(nt == 0), stop=(nt == NNT - 1), skip_group_check=True,
                )
            nc.vector.tensor_scalar_mul(pooledT_all[R, 0:384], pp0, rse[R])
            nc.vector.tensor_scalar_mul(pooledT_all[R, 384:768], pp1, rse[R])

        # ---- tail: out = Wp @ (Wv @ pooled)|diag + bpz ---------------------
        wv_sb = const.tile([128, CT, C], BF16)
        nc.sync.dma_start(wv_sb, wv_d.rearrange("(c p) o -> p c o", p=128))
        wp_sb = const.tile([128, CT, C], BF16)
        nc.sync.dma_start(wp_sb, wp_d.rearrange("(c p) o -> p c o", p=128))
        pooled_sb = const.tile([128, CT, BPC * HEADS], BF16)
        for c in range(CT):
            tpp = kp_pool.tile([128, 64], F32, tag="kp")
            nc.tensor.transpose(tpp, pooledT_all[:, ts(c, 128)], id64_f)
            for b in range(BPC):
                nc.vector.tensor_copy(
                    out=pooled_sb[:, c, b * HEADS : (b + 1) * HEADS],
                    in_=tpp[:, 32 * b : 32 * b + HEADS])

        outv_sb = const.tile([128, CT, BPC], BF16)
        for o in range(CT):
            vp = kp_pool.tile([128, BPC * HEADS], F32, tag="kp")
            for c in range(CT):
                nc.tensor.matmul(
                    vp, wv_sb[:, c, ts(o, 128)], pooled_sb[:, c, :],
                    start=(c == 0), stop=(c == CT - 1),
                )
            for half in range(2):
                h = 2 * o + half
                rows = slice(64 * half, 64 * (half + 1))
                for b in range(BPC):
                    col = b * HEADS + h
                    nc.vector.tensor_copy(
                        out=outv_sb[rows, o, b : b + 1],
                        in_=vp[rows, col : col + 1],
                    )

        z_sb = const.tile([128, CT, BPC], F32)
        for o2 in range(CT):
            zp = kp_pool.tile([128, BPC], F32, tag="kp")
            for o in range(CT):
                nc.tensor.matmul(
                    zp, wp_sb[:, o, ts(o2, 128)], outv_sb[:, o, :],
                    start=(o == 0), stop=(o == CT - 1),
                )
            nc.vector.tensor_tensor(
                out=z_sb[:, o2, :], in0=zp,
                in1=bpz_sb[:, o2, None].to_broadcast((128, BPC)), op=OP.add,
            )
        nc.sync.dma_start(z_d.rearrange("(c p) b -> p c b", p=128), z_sb)


_NC_CACHE = None


def _get_nc():
    global _NC_CACHE
    if _NC_CACHE is None:
        _NC_CACHE = _build_nc()
    return _NC_CACHE


def make_in_maps(inputs):
    x = np.ascontiguousarray(np.asarray(inputs["x"], dtype=np.float32)).reshape(B, C, N)
    y = np.asarray(inputs["y"], dtype=np.float32).reshape(B, C)
    Wq = np.asarray(inputs["Wq"], dtype=np.float32)
    bq = np.asarray(inputs["bq"], dtype=np.float32)
    Wkv = np.asarray(inputs["Wkv"], dtype=np.float32)
    bkv = np.asarray(inputs["bkv"], dtype=np.float32)
    Wp = np.asarray(inputs["Wp"], dtype=np.float32)
    bp = np.asarray(inputs["bp"], dtype=np.float32)

    wk, wv = Wkv[:C], Wkv[C:]
    bk, bv = bkv[:C], bkv[C:]
    wkT = np.ascontiguousarray(wk.T).astype(ml_dtypes.bfloat16)
    wk2 = np.ascontiguousarray(wk).astype(ml_dtypes.bfloat16)
    wqT = np.ascontiguousarray(Wq.T).astype(ml_dtypes.bfloat16)
    wvT = np.ascontiguousarray(wv.T).astype(ml_dtypes.bfloat16)
    wpT = np.ascontiguousarray(Wp.T).astype(ml_dtypes.bfloat16)
    bpz = (Wp @ bv + bp).astype(np.float32)

    in_maps = []
    for i in range(NCORES):
        in_maps.append({
            "x": np.ascontiguousarray(x[i * BPC : (i + 1) * BPC]),
            "y": np.ascontiguousarray(y[i * BPC : (i + 1) * BPC].T),
            "wkT": wkT, "wk2": wk2, "wqT": wqT, "wvT": wvT, "wpT": wpT,
            "bq": bq, "bk": np.ascontiguousarray(bk),
            "bpz": bpz,
        })
    return in_maps


def kernel(**inputs):
    nc = _get_nc()
    in_maps = make_in_maps(inputs)
    res = run_bass_kernel_spmd(nc, in_maps, core_ids=list(range(NCORES)))
    z = np.concatenate([r["z"].T for r in res.results], axis=0)
    return z.reshape(B, C, 1, 1).astype(np.float32)


# revision 18
# speedup vs baseline: 1.4590x; 1.4590x over previous
"""Trainium2 Bass kernel for nn_C_Cross_Attention3D (B=16, C=768, H=W=64, HEADS=12).

Math (per batch b):
  q   = l2norm_per_head(Wq @ y_b + bq)                      # [12, 64]
  k   = Wk @ x_b + bk                                       # [768, N], N = 4096
  s   = (Qbd^T k) / max(||k||_head, eps)                    # [12, N] cosine scores
  a   = softmax_N(s)                                        # [12, N]
  out = Wp @ (Wv @ (x_b @ a^T |head-diag) + bv) + bp        # [768]

Key restructuring vs. the reference: the V projection commutes with the
attention pooling (one query token per head), so instead of projecting all
N tokens through Wv we pool x with the attention weights first:
  out_attn[head h] = Wv[h_rows, :] @ (x @ a_h^T)  + bv
This halves the dominant GEMM (only K projection runs over all tokens).

Transposes (x^T for the pooling contraction, a^T) are done by DMA-transpose
through a DRAM bounce buffer in bf16, keeping the PE free for matmuls.

Distribution: pure data-parallel over batch, 2 batches per core, 8 cores.
No collectives; host scatters inputs / gathers outputs.

Self-contained: hardcodes all shapes; no sibling imports.
"""

import numpy as np
import ml_dtypes

import concourse.bass as bass
import concourse.mybir as mybir
import concourse.tile as tile
from concourse import bacc
from concourse.bass import ts
from concourse.bass_utils import run_bass_kernel_spmd
from concourse.masks import make_identity

F32 = mybir.dt.float32
BF16 = mybir.dt.bfloat16
AF = mybir.ActivationFunctionType
OP = mybir.AluOpType
AX = mybir.AxisListType

B, C, HEADS, HD = 16, 768, 12, 64
N = 64 * 64                 # tokens per batch
NCORES = 8
BPC = B // NCORES           # batches per core = 2
CT = C // 128               # 6 c-tiles (contraction / channel tiles)
FT = 512                    # token f-tile size
NFT = N // FT               # 8 f-tiles
NNT = N // 128              # 32 n-tiles of 128 tokens
NCH = 4                     # x^T DMA-transpose chunks per batch
CHW = N // NCH              # chunk width in tokens (1024)
EPS = 1e-12


def _act_table_filter():
    """Restrict activation-table choice to the single set that covers all
    funcs this kernel uses (Copy/Exp/Ln/Square), so no mid-kernel
    ACT_TABLE_LOAD swaps are emitted. Index positions are preserved."""
    import functools
    import concourse.bacc as _bacc

    orig = _bacc.get_activation_tables

    @functools.cache
    def filtered(arch):
        t = orig(arch)
        return {
            name: (s if name == "natural_log_exp_and_others" else set())
            for name, s in t.items()
        }

    return orig, filtered


def _build_nc():
    nc = bacc.Bacc(
        "TRN2",
        target_bir_lowering=False,
        debug=False,
        enable_asserts=False,
        num_devices=NCORES,
    )

    x_d = nc.dram_tensor("x", [BPC, C, N], F32, kind="ExternalInput").ap()
    y_d = nc.dram_tensor("y", [C, BPC], F32, kind="ExternalInput").ap()
    wk_d = nc.dram_tensor("wkT", [C, C], BF16, kind="ExternalInput").ap()
    wk2_d = nc.dram_tensor("wk2", [C, C], BF16, kind="ExternalInput").ap()
    wq_d = nc.dram_tensor("wqT", [C, C], BF16, kind="ExternalInput").ap()
    wv_d = nc.dram_tensor("wvT", [C, C], BF16, kind="ExternalInput").ap()
    wp_d = nc.dram_tensor("wpT", [C, C], BF16, kind="ExternalInput").ap()
    bq_d = nc.dram_tensor("bq", [C], F32, kind="ExternalInput").ap()
    bk_d = nc.dram_tensor("bk", [C], F32, kind="ExternalInput").ap()
    bpz_d = nc.dram_tensor("bpz", [C], F32, kind="ExternalInput").ap()
    z_d = nc.dram_tensor("z", [C, BPC], F32, kind="ExternalOutput").ap()

    with tile.TileContext(nc) as tc:
        _emit(nc, tc, x_d, y_d, wk_d, wk2_d, wq_d, wv_d, wp_d, bq_d, bk_d, bpz_d,
              z_d)
    import concourse.bacc as _bacc
    orig, filtered = _act_table_filter()
    _bacc.get_activation_tables = filtered
    try:
        nc.compile()
    finally:
        _bacc.get_activation_tables = orig
    return nc


def _emit(nc, tc, x_d, y_d, wk_d, wk2_d, wq_d, wv_d, wp_d, bq_d, bk_d, bpz_d,
          z_d):
    from contextlib import ExitStack

    ctx = ExitStack()
    with ctx:
        const = ctx.enter_context(tc.tile_pool(name="const", bufs=1))
        statics = ctx.enter_context(tc.tile_pool(name="statics", bufs=1))
        xf_pool = ctx.enter_context(tc.tile_pool(name="xf", bufs=2))
        xb_pool = ctx.enter_context(tc.tile_pool(name="xb", bufs=2))
        k2_pool = ctx.enter_context(tc.tile_pool(name="k2", bufs=2))
        small = ctx.enter_context(tc.tile_pool(name="small", bufs=4))
        at_pool = ctx.enter_context(tc.tile_pool(name="at", bufs=2))
        dram = ctx.enter_context(tc.tile_pool(name="dram", bufs=6, space="DRAM"))
        kp_pool = ctx.enter_context(tc.tile_pool(name="kp", bufs=6, space="PSUM"))
        pp_pool = ctx.enter_context(tc.tile_pool(name="pp", bufs=2, space="PSUM"))

        # ---- constants / weights -------------------------------------------
        id128_f = const.tile([128, 128], F32)
        make_identity(nc, id128_f)
        id64_f = const.tile([64, 64], F32)
        make_identity(nc, id64_f)

        wk_sb = const.tile([128, CT, C], BF16)
        nc.sync.dma_start(wk_sb, wk_d.rearrange("(c p) o -> p c o", p=128))
        wk2_sb = const.tile([128, CT, C], BF16)
        nc.sync.dma_start(wk2_sb, wk2_d.rearrange("(o p) c -> p o c", p=128))
        wq_sb = const.tile([128, CT, C], BF16)
        nc.sync.dma_start(wq_sb, wq_d.rearrange("(c p) o -> p c o", p=128))
        bq_sb = const.tile([128, CT], F32)
        nc.sync.dma_start(bq_sb, bq_d.rearrange("(c p) -> p c", p=128))
        bk_sb = const.tile([128, CT], F32)
        nc.sync.dma_start(bk_sb, bk_d.rearrange("(c p) -> p c", p=128))
        bpz_sb = const.tile([128, CT], F32)
        nc.sync.dma_start(bpz_sb, bpz_d.rearrange("(c p) -> p c", p=128))

        # ones_bd[c, h] = 1 if c // 64 == h  (block-diagonal head indicator)
        ones_bf = const.tile([128, CT, HEADS], BF16)
        ones_f = const.tile([128, CT, HEADS], F32)
        onesT_f = const.tile([HEADS, C], F32)
        nc.vector.memset(ones_bf, 0.0)
        nc.vector.memset(ones_f, 0.0)
        for c in range(CT):
            for half in range(2):
                h = 2 * c + half
                rows = slice(64 * half, 64 * (half + 1))
                nc.vector.memset(ones_bf[rows, c, h : h + 1], 1.0)
                nc.vector.memset(ones_f[rows, c, h : h + 1], 1.0)
        for c in range(CT):
            otp = kp_pool.tile([HEADS, 128], F32, tag="kp")
            nc.tensor.transpose(otp, ones_f[:, c, :], id128_f)
            nc.scalar.copy(out=onesT_f[:, ts(c, 128)], in_=otp)

        # ---- statics --------------------------------------------------------
        scores_all = statics.tile([64, N], F32)   # rows: 32*b + h (12 per batch)
        nc.vector.memset(scores_all, 0.0)
        xT_all = statics.tile([128, NNT, C], BF16)  # transposed x, current batch
        pooledT_all = statics.tile([64, C], F32)
        nc.vector.memset(pooledT_all, 0.0)

        # ---- q path (both batches at once) ---------------------------------
        y_sb = const.tile([128, CT, BPC], F32)
        nc.sync.dma_start(y_sb, y_d.rearrange("(c p) b -> p c b", p=128))
        y_bf = const.tile([128, CT, BPC], BF16)
        nc.vector.tensor_copy(out=y_bf, in_=y_sb)

        q_sb = const.tile([128, CT, BPC], F32)
        for o in range(CT):
            qp = kp_pool.tile([128, BPC], F32, tag="kp")
            for c in range(CT):
                nc.tensor.matmul(
                    qp, wq_sb[:, c, ts(o, 128)], y_bf[:, c, :],
                    start=(c == 0), stop=(c == CT - 1),
                )
            nc.vector.tensor_tensor(
                out=q_sb[:, o, :], in0=qp,
                in1=bq_sb[:, o, None].to_broadcast((128, BPC)), op=OP.add,
            )
        q2_sb = const.tile([128, CT, BPC], F32)
        nc.scalar.activation(out=q2_sb, in_=q_sb, func=AF.Square)
        ssqq = kp_pool.tile([HEADS, BPC], F32, tag="kp")
        for c in range(CT):
            nc.tensor.matmul(
                ssqq, ones_f[:, c, :], q2_sb[:, c, :],
                start=(c == 0), stop=(c == CT - 1),
            )
        rq = const.tile([HEADS, BPC], F32)
        nc.scalar.activation(out=rq, in_=ssqq, func=AF.Ln)
        nc.scalar.activation(out=rq, in_=rq, func=AF.Exp, scale=-0.5)
        nc.vector.tensor_scalar_min(rq, rq, 1.0 / EPS)
        # broadcast rq back to channel layout via block-diag ones matmul
        rqbc = kp_pool.tile([128, CT, BPC], F32, tag="kp")
        for c in range(CT):
            nc.tensor.matmul(
                rqbc[:, c, :], onesT_f[:, ts(c, 128)], rq,
                start=(c == 0), stop=(c == CT - 1), skip_group_check=True,
            )
        qn_sb = const.tile([128, CT, BPC], F32)
        nc.vector.tensor_tensor(out=qn_sb, in0=q_sb, in1=rqbc, op=OP.mult)
        # scatter into block-diagonal Qbd [c, 32*b + h]
        qbd_f = const.tile([128, CT, 32 * BPC], F32)
        nc.vector.memset(qbd_f, 0.0)
        for c in range(CT):
            for half in range(2):
                h = 2 * c + half
                rows = slice(64 * half, 64 * (half + 1))
                for b in range(BPC):
                    col = 32 * b + h
                    nc.vector.tensor_copy(
                        out=qbd_f[rows, c, col : col + 1],
                        in_=qn_sb[rows, c, b : b + 1],
                    )
        qbd_bf = const.tile([128, CT, 32 * BPC], BF16)
        nc.vector.tensor_copy(out=qbd_bf, in_=qbd_f)
        # fold q into the K projection: raw scores = (Wk^T Qbd)^T x + Qbd^T bk
        wtld_bf = const.tile([128, CT, 32 * BPC], BF16)
        for m in range(CT):
            wtp = kp_pool.tile([128, 32 * BPC], F32, tag="kp")
            for ot in range(CT):
                nc.tensor.matmul(
                    wtp, wk2_sb[:, ot, ts(m, 128)], qbd_bf[:, ot, :],
                    start=(ot == 0), stop=(ot == CT - 1),
                )
            nc.vector.tensor_copy(out=wtld_bf[:, m, :], in_=wtp)
        qbk_sb = const.tile([32 * BPC, 1], F32)
        qbkp = kp_pool.tile([32 * BPC, 1], F32, tag="kp")
        for ot in range(CT):
            nc.tensor.matmul(
                qbkp, qbd_f[:, ot, :], bk_sb[:, ot, None],
                start=(ot == 0), stop=(ot == CT - 1),
            )
        nc.vector.tensor_copy(out=qbk_sb, in_=qbkp)

        # ---- main per-batch ------------------------------------------------
        for b in range(BPC):
            R = slice(32 * b, 32 * b + HEADS)
            x_b = x_d[b].rearrange("(c p) n -> p c n", p=128)
            nmx8 = small.tile([64, NFT], F32, tag="nmx8")

            # bf16 bounce chunks in DRAM for the DMA-transpose of x
            xbd = [
                dram.tile([C, CHW], BF16, tag="xbd", name=f"xbd{b}_{t}")
                for t in range(NCH)
            ]

            for i in range(NFT):
                xf = xf_pool.tile([128, CT, FT], F32)
                nc.sync.dma_start(xf, x_b[:, :, ts(i, FT)])
                xb = xb_pool.tile([128, CT, FT], BF16)
                nc.vector.tensor_copy(out=xb, in_=xf)
                # write bf16 x to its bounce chunk (2 f-tiles per chunk)
                ch, off = divmod(i * FT, CHW)
                nc.sync.dma_start(
                    xbd[ch].rearrange("(c p) n -> p c n", p=128)[:, :, off : off + FT],
                    xb,
                )
                if off + FT == CHW:
                    # chunk complete -> transposed read into xT_all
                    nc.sync.dma_start_transpose(
                        xT_all[:, ch * (CHW // 128) : (ch + 1) * (CHW // 128), :],
                        xbd[ch][:],
                    )

                k2sb = k2_pool.tile([128, CT, FT], BF16)
                for o in range(CT):
                    kp = kp_pool.tile([128, FT], F32, tag="kp")
                    for c in range(CT):
                        nc.tensor.matmul(
                            kp, wk_sb[:, c, ts(o, 128)], xb[:, c, :],
                            start=(c == 0), stop=(c == CT - 1),
                        )
                    nc.scalar.activation(
                        out=k2sb[:, o, :], in_=kp, func=AF.Square,
                        bias=bk_sb[:, o : o + 1], scale=1.0,
                    )

                # sp on col groups 0-1, sq on col group 2 (partition base 64)
                # of a separate bank: interleaved matmuls run concurrently on
                # disjoint column groups of the PE array.
                spt = kp_pool.tile([32 * BPC, FT], F32, tag="kp")
                sqt = kp_pool.tile([64 + HEADS, FT], F32, tag="kp")
                sp = spt[:]
                sq = sqt[64 : 64 + HEADS]
                for c in range(CT):
                    nc.tensor.matmul(
                        sp, wtld_bf[:, c, :], xb[:, c, :],
                        start=(c == 0), stop=(c == CT - 1), skip_group_check=True,
                    )
                    nc.tensor.matmul(
                        sq, ones_bf[:, c, :], k2sb[:, c, :],
                        start=(c == 0), stop=(c == CT - 1), skip_group_check=True,
                    )
                # r = ssq^-1/2 (clamped to 1/eps); scores = (raw + qbk) * r
                rt = small.tile([HEADS, FT], F32, tag="rt")
                nc.scalar.activation(out=rt, in_=sq, func=AF.Ln)
                nc.scalar.activation(out=rt, in_=rt, func=AF.Exp, scale=-0.5)
                nc.vector.tensor_scalar_min(rt, rt, 1.0 / EPS)
                nc.vector.tensor_scalar(
                    out=sp[R, :], in0=sp[R, :],
                    scalar1=qbk_sb[R], scalar2=None, op0=OP.add,
                )
                nc.vector.tensor_tensor(
                    out=scores_all[R, ts(i, FT)], in0=sp[R, :], in1=rt, op=OP.mult,
                )
                nc.vector.tensor_reduce(
                    nmx8[R, i : i + 1], scores_all[R, ts(i, FT)],
                    axis=AX.X, op=OP.max)

            # softmax over N for this batch's 12 head-rows; exp goes straight
            # to the bf16 bounce tile, normalization is folded into the
            # pooled eviction (pooled /= sum(exp)).
            nmx = small.tile([64, 1], F32, tag="st")
            nc.vector.tensor_reduce(
                nmx[R], nmx8[R, :], axis=AX.X, op=OP.max, negate=True)
            se = small.tile([64, 1], F32, tag="se")
            nc.vector.memset(se[R], 0.0)
            ab = at_pool.tile([64, N], BF16, tag="ab")
            nc.scalar.activation(
                out=ab[R, :], in_=scores_all[R, :], func=AF.Exp,
                bias=nmx[R], scale=1.0, accum_out=se[R],
            )
            rse = small.tile([64, 1], F32, tag="st")
            nc.vector.reciprocal(rse[R], se[R])

            # attn -> DRAM -> transposed read  [128, NNT, 32]
            abd = dram.tile([32, N], BF16, tag="abd")
            nc.sync.dma_start(abd[:], ab[32 * b : 32 * b + 32, :])
            attnT = at_pool.tile([128, NNT, 32], BF16, tag="attnT")
            nc.sync.dma_start_transpose(attnT, abd[:])

            # attn-weighted pooling of x: pooledT = a @ x^T   [12, 768]
            ppt0 = pp_pool.tile([HEADS, 384], F32, tag="pp")
            ppt1 = pp_pool.tile([32 + HEADS, 384], F32, tag="pp")
            pp0 = ppt0[:]
            pp1 = ppt1[32 : 32 + HEADS]
            for nt in range(NNT):
                nc.tensor.matmul(
                    pp0, attnT[:, nt, 0:HEADS], xT_all[:, nt, 0:384],
                    start=(nt == 0), stop=(nt == NNT - 1), skip_group_check=True,
                )
                nc.tensor.matmul(
                    pp1, attnT[:, nt, 0:HEADS], xT_all[:, nt, 384:768],
                    start=(nt == 0), stop=(nt == NNT - 1), skip_group_check=True,
                )
            nc.vector.tensor_scalar_mul(pooledT_all[R, 0:384], pp0, rse[R])
            nc.vector.tensor_scalar_mul(pooledT_all[R, 384:768], pp1, rse[R])

        # ---- tail: out = Wp @ (Wv @ pooled)|diag + bpz ---------------------
        wv_sb = const.tile([128, CT, C], BF16)
        nc.sync.dma_start(wv_sb, wv_d.rearrange("(c p) o -> p c o", p=128))
        wp_sb = const.tile([128, CT, C], BF16)
        nc.sync.dma_start(wp_sb, wp_d.rearrange("(c p) o -> p c o", p=128))
        pooled_sb = const.tile([128, CT, BPC * HEADS], BF16)
        for c in range(CT):
            tpp = kp_pool.tile([128, 64], F32, tag="kp")
            nc.tensor.transpose(tpp, pooledT_all[:, ts(c, 128)], id64_f)
            for b in range(BPC):
                nc.vector.tensor_copy(
                    out=pooled_sb[:, c, b * HEADS : (b + 1) * HEADS],
                    in_=tpp[:, 32 * b : 32 * b + HEADS])

        outv_sb = const.tile([128, CT, BPC], BF16)
        for o in range(CT):
            vp = kp_pool.tile([128, BPC * HEADS], F32, tag="kp")
            for c in range(CT):
                nc.tensor.matmul(
                    vp, wv_sb[:, c, ts(o, 128)], pooled_sb[:, c, :],
                    start=(c == 0), stop=(c == CT - 1),
                )
            for half in range(2):
                h = 2 * o + half
                rows = slice(64 * half, 64 * (half + 1))
                for b in range(BPC):
                    col = b * HEADS + h
                    nc.vector.tensor_copy(
                        out=outv_sb[rows, o, b : b + 1],
                        in_=vp[rows, col : col + 1],
                    )

        z_sb = const.tile([128, CT, BPC], F32)
        for o2 in range(CT):
            zp = kp_pool.tile([128, BPC], F32, tag="kp")
            for o in range(CT):
                nc.tensor.matmul(
                    zp, wp_sb[:, o, ts(o2, 128)], outv_sb[:, o, :],
                    start=(o == 0), stop=(o == CT - 1),
                )
            nc.vector.tensor_tensor(
                out=z_sb[:, o2, :], in0=zp,
                in1=bpz_sb[:, o2, None].to_broadcast((128, BPC)), op=OP.add,
            )
        nc.sync.dma_start(z_d.rearrange("(c p) b -> p c b", p=128), z_sb)


_NC_CACHE = None


def _get_nc():
    global _NC_CACHE
    if _NC_CACHE is None:
        _NC_CACHE = _build_nc()
    return _NC_CACHE


def make_in_maps(inputs):
    x = np.ascontiguousarray(np.asarray(inputs["x"], dtype=np.float32)).reshape(B, C, N)
    y = np.asarray(inputs["y"], dtype=np.float32).reshape(B, C)
    Wq = np.asarray(inputs["Wq"], dtype=np.float32)
    bq = np.asarray(inputs["bq"], dtype=np.float32)
    Wkv = np.asarray(inputs["Wkv"], dtype=np.float32)
    bkv = np.asarray(inputs["bkv"], dtype=np.float32)
    Wp = np.asarray(inputs["Wp"], dtype=np.float32)
    bp = np.asarray(inputs["bp"], dtype=np.float32)

    wk, wv = Wkv[:C], Wkv[C:]
    bk, bv = bkv[:C], bkv[C:]
    wkT = np.ascontiguousarray(wk.T).astype(ml_dtypes.bfloat16)
    wk2 = np.ascontiguousarray(wk).astype(ml_dtypes.bfloat16)
    wqT = np.ascontiguousarray(Wq.T).astype(ml_dtypes.bfloat16)
    wvT = np.ascontiguousarray(wv.T).astype(ml_dtypes.bfloat16)
    wpT = np.ascontiguousarray(Wp.T).astype(ml_dtypes.bfloat16)
    bpz = (Wp @ bv + bp).astype(np.float32)

    in_maps = []
    for i in range(NCORES):
        in_maps.append({
            "x": np.ascontiguousarray(x[i * BPC : (i + 1) * BPC]),
            "y": np.ascontiguousarray(y[i * BPC : (i + 1) * BPC].T),
            "wkT": wkT, "wk2": wk2, "wqT": wqT, "wvT": wvT, "wpT": wpT,
            "bq": bq, "bk": np.ascontiguousarray(bk),
            "bpz": bpz,
        })
    return in_maps


def kernel(**inputs):
    nc = _get_nc()
    in_maps = make_in_maps(inputs)
    res = run_bass_kernel_spmd(nc, in_maps, core_ids=list(range(NCORES)))
    z = np.concatenate([r["z"].T for r in res.results], axis=0)
    return z.reshape(B, C, 1, 1).astype(np.float32)
